# revision 1
# baseline (speedup 1.0000x reference)
"""Trainium2 Bass kernel for nn_CapgMyoNet (dense CNN), 8-core data-parallel.

Network (per sample): permute(8,16) -> bn0 -> conv3x3(1->64)+bn+relu
  -> conv3x3(64->64)+bn+relu -> 2x locally-connected 1x1 (per-pixel 64x64)
  -> fc 8192->512 -> fc 512->512 -> fc 512->128 -> fc 128->8
All bn folded into weights/biases on host. bf16 matmuls, fp32 accumulate.

Mapping (per core, 1024 samples):
- conv1 as dense position-matmul: out{p,p}-dup chunks [K=128 raw pix, M=128],
  folds the input permutation, bn0 and SAME-padding into the host matrix M1.
- conv2: offset-pair row-packed matmuls (tile_position), 2 sample-quads
  col-packed per PSUM bank; border offsets use sub-rectangle strided APs.
- lc3+lc4 fused per pixel-pair chunk, 4-way 64x64 tile-packing.
- fc5 accumulated on the fly from lc4 output chunks (weights streamed).
- fc6/7/8 straightforward K-chunked matmuls.
Activations flow lower/upper-partition-half sample-split; host unpermutes
the resulting column order at the end.
"""
import numpy as np
import ml_dtypes

import concourse.bass as bass
import concourse.bacc as bacc
import concourse.mybir as mybir
import concourse.tile as tile

bf16 = mybir.dt.bfloat16
f32 = mybir.dt.float32

H, W, C, NCLS = 8, 16, 64, 8
NPOS = H * W  # 128
EPS = 1e-5
NCORES = 8

# conv2 offset pairing: rounds of (lower-strip offset, upper-strip offset).
# center offset (4) first: it is full-coverage so start=True covers the bank.
OSEQ = [4, 0, 1, 2, 3, 5, 6, 7, 8]


def _valid_box(o9):
    ky, kx = o9 // 3, o9 % 3
    dy, dx = ky - 1, kx - 1
    y0 = max(0, -dy); y1 = H + min(0, -dy)
    x0 = max(0, -dx); x1 = W + min(0, -dx)
    return dy, dx, y0, y1, x0, x1


def build(NB=1024, PASSN=128, debug_taps=False):
    """Build the per-core bass program. NB = samples per core."""
    assert PASSN % 8 == 0 and NB % (2 * PASSN) == 0
    NPASS = NB // PASSN
    GROUP = 2 * PASSN
    NG = NB // GROUP
    HALFG = PASSN            # A3 cols per partition half (per group)
    QP = PASSN // 8          # conv2 quad-pairs per pass
    NSPL = max(1, NB // 512)  # fc6/7/8 column splits
    NCOLS = NB // NSPL

    nc = bacc.Bacc("TRN2", target_bir_lowering=False, debug=False)
    ev_ct = [0]

    def evict(out_ap, in_ap, bias_ap, relu=True):
        """Alternating-engine psum->sbuf eviction with bias (+relu)."""
        ev_ct[0] += 1
        if ev_ct[0] % 2 == 0:
            if relu:
                nc.scalar.activation(out_ap, in_ap,
                                     mybir.ActivationFunctionType.Relu,
                                     bias=bias_ap)
            else:
                nc.vector.tensor_scalar(out_ap, in_ap, bias_ap, None,
                                        mybir.AluOpType.add)
        else:
            if relu:
                nc.vector.tensor_scalar(out_ap, in_ap, bias_ap, 0.0,
                                        mybir.AluOpType.add,
                                        mybir.AluOpType.max)
            else:
                nc.vector.tensor_scalar(out_ap, in_ap, bias_ap, None,
                                        mybir.AluOpType.add)

    with tile.TileContext(nc) as tc:
        from contextlib import ExitStack
        es = ExitStack()
        with es:
            dram = es.enter_context(tc.tile_pool(name="dram", bufs=1, space="DRAM"))
            wp = es.enter_context(tc.tile_pool(name="wp", bufs=1))
            big = es.enter_context(tc.tile_pool(name="big", bufs=1))
            ring = es.enter_context(tc.tile_pool(name="ring", bufs=3))

            # ---------------- DRAM I/O ----------------
            x_d = dram.tile([NB, 128], f32, kind="ExternalInput", name="x", uniquify=False)
            m1_d = dram.tile([128, NPOS * 64], bf16, kind="ExternalInput", name="m1", uniquify=False)
            w2_d = dram.tile([128, 9 * 64], bf16, kind="ExternalInput", name="w2", uniquify=False)
            w3_d = dram.tile([128, NPOS * 64], bf16, kind="ExternalInput", name="w3", uniquify=False)
            w4_d = dram.tile([128, NPOS * 64], bf16, kind="ExternalInput", name="w4", uniquify=False)
            fc5w_d = dram.tile([64, 128, 512], bf16, kind="ExternalInput", name="fc5w", uniquify=False)
            fc6w_d = dram.tile([128, 16 * 128], bf16, kind="ExternalInput", name="fc6w", uniquify=False)
            fc78w_d = dram.tile([128, 4 * 128 + NCLS], bf16, kind="ExternalInput", name="fc78w", uniquify=False)
            consts_d = dram.tile([128, 16], f32, kind="ExternalInput", name="consts", uniquify=False)
            y_d = dram.tile([NCLS, NB], f32, kind="ExternalOutput", name="y", uniquify=False)
            if debug_taps:
                dbg_a2_d = dram.tile([128, PASSN * NPOS], f32, kind="ExternalOutput", name="dbg_a2", uniquify=False)
                dbg_a3_d = dram.tile([128, NPOS * GROUP], f32, kind="ExternalOutput", name="dbg_a3", uniquify=False)
                dbg_f6_d = dram.tile([128, 4 * NB], f32, kind="ExternalOutput", name="dbg_f6", uniquify=False)

            # ---------------- persistent SBUF ----------------
            consts = wp.tile([128, 16], f32, name="consts_sb")
            nc.sync.dma_start(out=consts[:], in_=consts_d[:])
            B1 = consts[:, 0:1]; B2 = consts[:, 1:2]
            B3 = consts[:, 2:3]; B4 = consts[:, 3:4]
            B5 = [consts[:, 4 + m:5 + m] for m in range(4)]
            B6 = [consts[:, 8 + m:9 + m] for m in range(4)]
            B7 = consts[:, 12:13]
            S0 = consts[:, 13:14]; T0 = consts[:, 14:15]
            B8 = consts[0:NCLS, 15:16]

            w2_sb = wp.tile([128, 9 * 64], bf16, name="w2_sb", tag="w2ovl")
            nc.sync.dma_start(out=w2_sb[:], in_=w2_d[:])
            m1_sb = wp.tile([128, NPOS * 64], bf16, name="m1_sb", tag="m1ovl")
            nc.sync.dma_start(out=m1_sb[:], in_=m1_d[:])
            w3_sb = wp.tile([128, NPOS * 64], bf16, name="w3_sb", tag="w3ovl")
            nc.sync.dma_start(out=w3_sb[:], in_=w3_d[:])
            w4_sb = wp.tile([128, NPOS * 64], bf16, name="w4_sb", tag="w4ovl")
            nc.sync.dma_start(out=w4_sb[:], in_=w4_d[:])

            ident = wp.tile([128, 128], f32, name="ident")
            from concourse.masks import make_identity
            make_identity(nc, ident[:])

            xT = wp.tile([128, NB], bf16, name="xT", tag="xTovl")
            F6 = wp.tile([128, 4 * NB], bf16, name="F6")
            A2 = big.tile([128, PASSN * NPOS], bf16, name="A2")
            A3 = big.tile([128, NPOS * GROUP], bf16, name="A3")

            # ---------------- P0: load + transpose + bn0 ----------------
            with tc.tile_pool(name="p0ps", bufs=2, space="PSUM") as p0ps, \
                 tc.tile_pool(name="xstp", bufs=2) as xstp:
                ntile = (NB + 127) // 128
                for t in range(ntile):
                    n0 = t * 128
                    nn = min(128, NB - n0)
                    xst = xstp.tile([128, 128], f32, name="xst", tag="xst")
                    nc.sync.dma_start(out=xst[0:nn, :], in_=x_d[n0:n0 + nn, :])
                    ps0 = p0ps.tile([128, 128], f32, name="ps0", tag="ps0")
                    nc.tensor.matmul(ps0[:, 0:nn], xst[0:nn, :], ident[0:nn, 0:nn],
                                     is_transpose=True, start=True, stop=True,
                                     skip_group_check=True)
                    # xT = s0 * x^T + t0  (bn0; single channel so scalars)
                    nc.vector.tensor_scalar(xT[:, n0:n0 + nn], ps0[:, 0:nn], S0, T0,
                                            mybir.AluOpType.mult, mybir.AluOpType.add)

            a2v = A2[:].rearrange("q (s y x) -> q s y x", s=PASSN, y=H, x=W)
            a2pv = A2[:].rearrange("q (s p) -> q p s", s=PASSN, p=NPOS)
            a3kv = A3[:].rearrange("q (p k j) -> q k j p", p=NPOS, k=GROUP // 4, j=4)
            a3pix = A3[:].rearrange("q (p c) -> q p c", p=NPOS, c=GROUP)
            f6v = F6[:].rearrange("q (m n) -> q m n", m=4, n=NB)

            for g in range(NG):
                # ============ conv passes (2 per group) ============
                for pb in range(2):
                    pi = 2 * g + pb
                    nb0 = pi * PASSN
                    # ---- conv1: dense position matmuls into A2 ----
                    with tc.tile_pool(name=f"c1ps{pi}", bufs=2, space="PSUM") as c1pp:
                        for b0 in range(0, NPOS, 4):
                            psC1 = c1pp.tile([128, 4 * PASSN], f32, name="psC1", tag="psC1")
                            for i in range(4):
                                p = b0 + i
                                wsl = m1_sb[:, 64 * p:64 * p + 64]
                                cb = i * PASSN
                                # duplicated channels on both partition halves via
                                # two concurrent col-split matmuls (same weights)
                                nc.tensor.matmul(
                                    psC1[0:64, cb:cb + PASSN], wsl, xT[:, nb0:nb0 + PASSN],
                                    start=True, stop=True, tile_position=(0, 0),
                                    skip_group_check=True)
                                nc.tensor.matmul(
                                    psC1[64:128, cb:cb + PASSN], wsl, xT[:, nb0:nb0 + PASSN],
                                    start=True, stop=True, tile_position=(0, 64),
                                    skip_group_check=True)
                            pc1 = psC1[:].rearrange("q (c s) -> q c s", c=4, s=PASSN)
                            evict(a2pv[:, b0:b0 + 4, :], pc1[:, :, :], B1)
                    # ---- conv2: into A3 (all matmul inputs on partitions 0-63) ----
                    with tc.tile_pool(name=f"c2ps{pi}", bufs=2, space="PSUM") as c2pp:
                        for kk in range(QP // 2):
                            psC2 = c2pp.tile([128, 1024], f32, name="psC2", tag="psC2")
                            for sub in range(2):
                                k = 2 * kk + sub
                                cb = 512 * sub
                                for r, o9 in enumerate(OSEQ):
                                    dy, dx, y0, y1, x0, x1 = _valid_box(o9)
                                    wt = w2_sb[0:64, 64 * o9:64 * o9 + 64]
                                    for cg, s0 in ((0, 4 * k), (64, PASSN // 2 + 4 * k)):
                                        rhs = a2v[0:64, s0:s0 + 4,
                                                  y0 + dy:y1 + dy, x0 + dx:x1 + dx]
                                        outap = psC2[cg:cg + 64, cb:cb + 512].rearrange(
                                            "q (s y x) -> q s y x", s=4, y=H, x=W)[:, :, y0:y1, x0:x1]
                                        nc.tensor.matmul(
                                            outap, wt, rhs,
                                            start=(r == 0), stop=(r == 8),
                                            tile_position=(0, cg),
                                            skip_group_check=True)
                            # evict: quad-lo (psum[0:64]) -> A3 cols = its sample ids,
                            # quad-hi (psum[64:128]) -> cross to lower partitions.
                            pc2 = psC2[:].rearrange("q (u s p) -> q u s p", u=2, s=4, p=NPOS)
                            for half in range(2):
                                c0 = PASSN * pb + (PASSN // 2) * half + 8 * kk
                                evict(a3kv[0:64, (c0 // 4):(c0 // 4) + 2, :, :],
                                      pc2[64 * half:64 * half + 64, :, :, :], B2[0:64])
                    if debug_taps and pi == NPASS - 1:
                        dbgt = wp.tile([128, PASSN * NPOS], f32, name="dbg_a2sb")
                        nc.any.tensor_copy(dbgt[:], A2[:])
                        nc.sync.dma_start(out=dbg_a2_d[:], in_=dbgt[:])

                if debug_taps and g == 0:
                    dbgt3 = wp.tile([128, NPOS * GROUP], f32, name="dbg_a3sb")
                    nc.any.memset(dbgt3[:], 0.0)
                    nc.any.tensor_copy(dbgt3[0:64, :], A3[0:64, :])
                    nc.sync.dma_start(out=dbg_a3_d[:], in_=dbgt3[:])

                # ============ lc3 + lc4 + fc5 fused over pixel-pair chunks ============
                with tc.tile_pool(name=f"fc5ps{g}", bufs=1, space="PSUM") as f5pp, \
                     tc.tile_pool(name=f"lcps{g}", bufs=2, space="PSUM") as lcpp, \
                     tc.tile_pool(name=f"lcsb{g}", bufs=3) as lcsb:
                    ps5 = [f5pp.tile([128, GROUP], f32, name=f"ps5_{m}", tag=f"ps5_{m}")
                           for m in range(4)]
                    for j in range(64):
                        px0, px1 = 2 * j, 2 * j + 1
                        ps3 = lcpp.tile([128, GROUP], f32, name="ps3", tag="ps3")
                        nc.tensor.matmul(ps3[0:64, :], w3_sb[0:64, 64 * px0:64 * px0 + 64],
                                         a3pix[0:64, px0, :], start=True, stop=True,
                                         tile_position=(0, 0), skip_group_check=True)
                        nc.tensor.matmul(ps3[64:128, :], w3_sb[0:64, 64 * px1:64 * px1 + 64],
                                         a3pix[0:64, px1, :], start=True, stop=True,
                                         tile_position=(0, 64), skip_group_check=True)
                        # evict both pixels to lower partitions, side by side
                        tmp = lcsb.tile([128, 2 * GROUP], bf16, name="tmp", tag="tmp")
                        evict(tmp[0:64, 0:GROUP], ps3[0:64, :], B3[0:64])
                        evict(tmp[0:64, GROUP:2 * GROUP], ps3[64:128, :], B3[0:64])
                        ps4 = lcpp.tile([128, GROUP], f32, name="ps4", tag="ps4")
                        nc.tensor.matmul(ps4[0:64, :], w4_sb[0:64, 64 * px0:64 * px0 + 64],
                                         tmp[0:64, 0:GROUP], start=True, stop=True,
                                         tile_position=(0, 0), skip_group_check=True)
                        nc.tensor.matmul(ps4[64:128, :], w4_sb[0:64, 64 * px1:64 * px1 + 64],
                                         tmp[0:64, GROUP:2 * GROUP], start=True, stop=True,
                                         tile_position=(0, 64), skip_group_check=True)
                        Fj = lcsb.tile([128, GROUP], bf16, name="Fj", tag="Fj")
                        evict(Fj[:], ps4[:], B4)
                        wst = lcsb.tile([128, 512], bf16, name="wst", tag="wst")
                        nc.sync.dma_start(out=wst[:], in_=fc5w_d[j])
                        for m in range(4):
                            nc.tensor.matmul(ps5[m][:, :], wst[:, 128 * m:128 * m + 128],
                                             Fj[:, :], start=(j == 0), stop=(j == 63))
                    for m in range(4):
                        evict(f6v[:, m, g * GROUP:(g + 1) * GROUP], ps5[m][:, :], B5[m])

            # ============ fc6 / fc7 / fc8 ============
            fc6w_sb = wp.tile([128, 16 * 128], bf16, name="fc6w_sb", tag="w3ovl")
            nc.sync.dma_start(out=fc6w_sb[:], in_=fc6w_d[:])
            fc78w_sb = wp.tile([128, 4 * 128 + NCLS], bf16, name="fc78w_sb", tag="xTovl")
            nc.sync.dma_start(out=fc78w_sb[:], in_=fc78w_d[:])
            F7 = wp.tile([128, 4 * NB], bf16, name="F7", tag="m1ovl")
            F8 = wp.tile([128, NB], bf16, name="F8", tag="w2ovl")
            y_sb = wp.tile([NCLS, NB], f32, name="y_sb", tag="w4ovl")
            f7v = F7[:].rearrange("q (m n) -> q m n", m=4, n=NB)

            if debug_taps:
                dbg6 = wp.tile([128, 4 * NB], f32, name="dbg_f6sb")
                nc.any.tensor_copy(dbg6[:], F6[:])
                nc.sync.dma_start(out=dbg_f6_d[:], in_=dbg6[:])

            with tc.tile_pool(name="fcps", bufs=4, space="PSUM") as fcpp, \
                 tc.tile_pool(name="fc8ps", bufs=2, space="PSUM") as fc8pp:
                for n in range(NSPL):
                    n0 = n * NCOLS
                    for m in range(4):
                        ps6 = fcpp.tile([128, NCOLS], f32, name="ps6", tag="ps6")
                        for jj in range(4):
                            nc.tensor.matmul(ps6[:, :],
                                             fc6w_sb[:, (4 * jj + m) * 128:(4 * jj + m) * 128 + 128],
                                             f6v[:, jj, n0:n0 + NCOLS],
                                             start=(jj == 0), stop=(jj == 3))
                        evict(f7v[:, m, n0:n0 + NCOLS], ps6[:, :], B6[m])
                for n in range(NSPL):
                    n0 = n * NCOLS
                    ps7 = fcpp.tile([128, NCOLS], f32, name="ps7", tag="ps6")
                    for jj in range(4):
                        nc.tensor.matmul(ps7[:, :],
                                         fc78w_sb[:, 128 * jj:128 * jj + 128],
                                         f7v[:, jj, n0:n0 + NCOLS],
                                         start=(jj == 0), stop=(jj == 3))
                    evict(F8[:, n0:n0 + NCOLS], ps7[:, :], B7)
                for n in range(NSPL):
                    n0 = n * NCOLS
                    ps8 = fc8pp.tile([NCLS, NCOLS], f32, name="ps8", tag="ps8")
                    nc.tensor.matmul(ps8[:, :], fc78w_sb[:, 512:512 + NCLS],
                                     F8[:, n0:n0 + NCOLS], start=True, stop=True)
                    nc.vector.tensor_scalar(y_sb[:, n0:n0 + NCOLS], ps8[:, :], B8, None,
                                            mybir.AluOpType.add)
            nc.sync.dma_start(out=y_d[:], in_=y_sb[:])

    nc.compile()
    return nc


# ---------------------------------------------------------------------------
# host-side weight preparation
# ---------------------------------------------------------------------------

def _bn_affine(p):
    g, b, m, v = p[0], p[1], p[2], p[3]
    s = g / np.sqrt(v + EPS)
    return s.astype(np.float32), (b - m * s).astype(np.float32)


def prep_weights(inputs):
    bf = ml_dtypes.bfloat16
    s0, t0 = _bn_affine(inputs['bn0']); s0, t0 = float(s0[0]), float(t0[0])
    s1, t1 = _bn_affine(inputs['bn1'])
    s2, t2 = _bn_affine(inputs['bn2'])
    s3, t3 = _bn_affine(inputs['bn3'])
    s4, t4 = _bn_affine(inputs['bn4'])
    s5, t5 = _bn_affine(inputs['bn5']); s5, t5 = float(s5[0]), float(t5[0])
    s6, t6 = _bn_affine(inputs['bn6']); s6, t6 = float(s6[0]), float(t6[0])
    s7, t7 = _bn_affine(inputs['bn7']); s7, t7 = float(s7[0]), float(t7[0])

    w1 = np.asarray(inputs['conv1_w'], np.float32)      # [64,1,3,3]
    m1 = np.zeros((128, NPOS * 64), np.float32)
    for p in range(NPOS):
        py, px = p // W, p % W
        for ky in range(3):
            for kx in range(3):
                iy, jx = py + ky - 1, px + kx - 1
                if 0 <= iy < H and 0 <= jx < W:
                    praw = 8 * jx + iy
                    m1[praw, 64 * p:64 * p + 64] += s1 * w1[:, 0, ky, kx]
    bias1 = (s1 * np.asarray(inputs['conv1_b'], np.float32) + t1)

    w2r = np.asarray(inputs['conv2_w'], np.float32)     # [64,64,3,3]
    w2 = np.zeros((128, 9 * 64), np.float32)
    for o9 in range(9):
        ky, kx = o9 // 3, o9 % 3
        blk = (s2[None, :] * w2r[:, :, ky, kx].T)       # [cin, cout]
        w2[0:64, 64 * o9:64 * o9 + 64] = blk
        w2[64:128, 64 * o9:64 * o9 + 64] = blk
    bias2 = (s2 * np.asarray(inputs['conv2_b'], np.float32) + t2)

    def lc(wname, s):
        wr = np.asarray(inputs[wname], np.float32)      # [o, c, h, w]
        out = np.zeros((128, NPOS * 64), np.float32)
        for p in range(NPOS):
            blk = s[None, :] * wr[:, :, p // W, p % W].T   # [cin, cout]
            out[0:64, 64 * p:64 * p + 64] = blk
            out[64:128, 64 * p:64 * p + 64] = blk
        return out
    w3 = lc('lc3_w', s3)
    w4 = lc('lc4_w', s4)

    fc5 = np.asarray(inputs['fc5_w'], np.float32)       # [512, 8192]
    fc5w = np.zeros((64, 128, 512), np.float32)
    # kpart<64 -> ch=kpart pixel 2j ; >=64 -> ch-64 pixel 2j+1
    for j in range(64):
        for half in range(2):
            ch_idx = np.arange(64)
            flat = 128 * ch_idx + (2 * j + half)
            fc5w[j, 64 * half:64 * half + 64, :] = s5 * fc5[:, flat].T
    bias5 = s5 * np.asarray(inputs['fc5_b'], np.float32) + t5   # [512]

    fc6 = np.asarray(inputs['fc6_w'], np.float32)       # [512, 512]
    fc6w = np.zeros((128, 16 * 128), np.float32)
    for jj in range(4):
        for m in range(4):
            blk = s6 * fc6[128 * m:128 * m + 128, 128 * jj:128 * jj + 128].T
            fc6w[:, (4 * jj + m) * 128:(4 * jj + m) * 128 + 128] = blk
    bias6 = s6 * np.asarray(inputs['fc6_b'], np.float32) + t6   # [512]

    fc7 = np.asarray(inputs['fc7_w'], np.float32)       # [128, 512]
    fc78w = np.zeros((128, 4 * 128 + NCLS), np.float32)
    for jj in range(4):
        fc78w[:, 128 * jj:128 * jj + 128] = s7 * fc7[:, 128 * jj:128 * jj + 128].T
    bias7 = s7 * np.asarray(inputs['fc7_b'], np.float32) + t7   # [128]
    fc8 = np.asarray(inputs['fc8_w'], np.float32)       # [8, 128]
    fc78w[:, 512:512 + NCLS] = fc8.T
    bias8 = np.asarray(inputs['fc8_b'], np.float32)     # [8]

    consts = np.zeros((128, 16), np.float32)
    consts[:, 0] = np.concatenate([bias1, bias1])
    consts[:, 1] = np.concatenate([bias2, bias2])
    consts[:, 2] = np.concatenate([t3, t3])
    consts[:, 3] = np.concatenate([t4, t4])
    for m in range(4):
        consts[:, 4 + m] = bias5[128 * m:128 * m + 128]
        consts[:, 8 + m] = bias6[128 * m:128 * m + 128]
    consts[:, 12] = bias7
    consts[:, 13] = s0
    consts[:, 14] = t0
    consts[0:NCLS, 15] = bias8

    return {
        'm1': m1.astype(bf), 'w2': w2.astype(bf), 'w3': w3.astype(bf),
        'w4': w4.astype(bf), 'fc5w': fc5w.astype(bf), 'fc6w': fc6w.astype(bf),
        'fc78w': fc78w.astype(bf), 'consts': consts,
    }


def sample_perm(NB, PASSN):
    """A3/F columns now carry group samples in identity order."""
    return np.arange(NB, dtype=np.int64)


_cache = {}


def _get_nc(NB=1024, PASSN=128, debug_taps=False):
    key = (NB, PASSN, debug_taps)
    if key not in _cache:
        _cache[key] = build(NB, PASSN, debug_taps)
    return _cache[key]


def kernel(**inputs):
    from concourse.bass_utils import run_bass_kernel_spmd
    x = np.asarray(inputs['x'], np.float32)
    B = x.shape[0]
    NB = B // NCORES
    xf = x.reshape(B, 128)
    w = prep_weights(inputs)
    nc = _get_nc(NB=NB, PASSN=128)
    in_maps = []
    for c in range(NCORES):
        m = dict(w)
        m['x'] = np.ascontiguousarray(xf[c * NB:(c + 1) * NB])
        in_maps.append(m)
    res = run_bass_kernel_spmd(nc, in_maps, list(range(NCORES)))
    return _assemble(res, B, NB)


def _assemble(res, B, NB):
    perm = sample_perm(NB, 128)
    out = np.empty((B, NCLS), np.float32)
    for c in range(NCORES):
        yc = np.asarray(res.results[c]['y'], np.float32)   # [8, NB]
        out[c * NB + perm] = yc.T
    return out


def run_traced(inputs, tmpdir=None):
    """Like kernel() but with NTFF tracing; returns (out, BassKernelResults)."""
    from concourse.bass_utils import run_bass_kernel_spmd
    x = np.asarray(inputs['x'], np.float32)
    B = x.shape[0]
    NB = B // NCORES
    xf = x.reshape(B, 128)
    w = prep_weights(inputs)
    nc = _get_nc(NB=NB, PASSN=128)
    in_maps = []
    for c in range(NCORES):
        m = dict(w)
        m['x'] = np.ascontiguousarray(xf[c * NB:(c + 1) * NB])
        in_maps.append(m)
    res = run_bass_kernel_spmd(nc, in_maps, list(range(NCORES)), trace=True,
                               tmpdir=tmpdir)
    return _assemble(res, B, NB), res



# revision 2
# speedup vs baseline: 3.2144x; 3.2144x over previous
"""Trainium2 Bass kernel for nn_CapgMyoNet (dense CNN), 8-core data-parallel.

Network (per sample): permute(8,16) -> bn0 -> conv3x3(1->64)+bn+relu
  -> conv3x3(64->64)+bn+relu -> 2x locally-connected 1x1 (per-pixel 64x64)
  -> fc 8192->512 -> fc 512->512 -> fc 512->128 -> fc 128->8
All bn folded into weights/biases on host. bf16 matmuls, fp32 accumulate.

Layout: activations live as [128 partitions = 64ch x row-parity], columns =
(ypair, x, sample) over a zero-padded slot grid (6 ypairs x 18 x incl pads).
- conv1: K=128 raw-pixel dense matmul, M=128 = two row-parity pixels packed.
- conv2: 9 taps = pure column-shift matmuls accumulated in PSUM; chunks
  alternate normal/parity-swapped output column groups so 4 independent
  64x64 quadrant matmuls run concurrently (full PE array).
- lc3/lc4: per-slot 64x64 quadrant matmuls, slot pairs pack 4 quadrants.
- fc5: K-chunk = one slot's 128 partitions; 4 M-chunks accumulate per group.
- fc6/7/8: straightforward K-chunked matmuls, N=512.
All PSUM->SBUF evictions are contiguous [128,512] with bias+relu fused.
"""
import numpy as np
import ml_dtypes

import concourse.bass as bass
import concourse.bacc as bacc
import concourse.mybir as mybir
import concourse.tile as tile

bf16 = mybir.dt.bfloat16
f32 = mybir.dt.float32

H, W, C, NCLS = 8, 16, 64, 8
EPS = 1e-5
NCORES = 8

YP = 4       # real ypair rows
XS = 18      # x slots incl 1 pad each side
YS = 6       # ypair slots incl 1 pad each side
NSLOT = YP * W  # 64 real slots


def sig1(Y, x):
    """Parity-swap flag of A3 slot (Y, x): conv2 chunk role."""
    return (Y * 8 + x // 2) % 2


def build(NB=1024, S=256, debug_taps=False):
    """Per-core program. NB samples/core, S samples per conv pass."""
    NPASS = NB // S
    G = 2 * S           # lc/fc5 group size
    NG = NB // G
    NSPL = max(1, NB // 512)
    NCOLS = NB // NSPL

    nc = bacc.Bacc("TRN2", target_bir_lowering=False, debug=False)
    ev_ct = [0]

    def evict(out_ap, in_ap, bias_ap, relu=True):
        """Alternating-engine psum->sbuf eviction with bias (+relu)."""
        ev_ct[0] += 1
        if ev_ct[0] % 2 == 0:
            if relu:
                nc.scalar.activation(out_ap, in_ap,
                                     mybir.ActivationFunctionType.Relu,
                                     bias=bias_ap)
            else:
                nc.vector.tensor_scalar(out_ap, in_ap, bias_ap, None,
                                        mybir.AluOpType.add)
        else:
            if relu:
                nc.vector.tensor_scalar(out_ap, in_ap, bias_ap, 0.0,
                                        mybir.AluOpType.add,
                                        mybir.AluOpType.max)
            else:
                nc.vector.tensor_scalar(out_ap, in_ap, bias_ap, None,
                                        mybir.AluOpType.add)

    def a2c(Y, x):
        """A2 column base of slot (ypair Y, x)."""
        return ((Y + 1) * XS + (x + 1)) * S

    with tile.TileContext(nc) as tc:
        from contextlib import ExitStack
        es = ExitStack()
        with es:
            dram = es.enter_context(tc.tile_pool(name="dram", bufs=1, space="DRAM"))
            wp = es.enter_context(tc.tile_pool(name="wp", bufs=1))
            big = es.enter_context(tc.tile_pool(name="big", bufs=1))

            # ---------------- DRAM I/O ----------------
            x_d = dram.tile([NB, 128], f32, kind="ExternalInput", name="x", uniquify=False)
            m1_d = dram.tile([128, NSLOT * 128], bf16, kind="ExternalInput", name="m1", uniquify=False)
            w2_d = dram.tile([128, 9 * 64], bf16, kind="ExternalInput", name="w2", uniquify=False)
            w3_d = dram.tile([128, NSLOT * 64], bf16, kind="ExternalInput", name="w3", uniquify=False)
            w4_d = dram.tile([128, NSLOT * 64], bf16, kind="ExternalInput", name="w4", uniquify=False)
            fc5w_d = dram.tile([NSLOT, 128, 512], bf16, kind="ExternalInput", name="fc5w", uniquify=False)
            fc6w_d = dram.tile([128, 16 * 128], bf16, kind="ExternalInput", name="fc6w", uniquify=False)
            fc78w_d = dram.tile([128, 4 * 128 + NCLS], bf16, kind="ExternalInput", name="fc78w", uniquify=False)
            consts_d = dram.tile([128, 16], f32, kind="ExternalInput", name="consts", uniquify=False)
            y_d = dram.tile([NCLS, NB], f32, kind="ExternalOutput", name="y", uniquify=False)
            if debug_taps:
                dbg_a2_d = dram.tile([128, YS * XS * S], bf16, kind="ExternalOutput", name="dbg_a2", uniquify=False)
                dbg_a3_d = dram.tile([128, NSLOT * G], bf16, kind="ExternalOutput", name="dbg_a3", uniquify=False)
                dbg_f6_d = dram.tile([128, 4 * NB], bf16, kind="ExternalOutput", name="dbg_f6", uniquify=False)

            # ---------------- persistent SBUF ----------------
            consts = wp.tile([128, 16], f32, name="consts_sb")
            nc.sync.dma_start(out=consts[:], in_=consts_d[:])
            B1 = consts[:, 0:1]; B2 = consts[:, 1:2]
            B3 = consts[:, 2:3]; B4 = consts[:, 3:4]
            B5 = [consts[:, 4 + m:5 + m] for m in range(4)]
            B6 = [consts[:, 8 + m:9 + m] for m in range(4)]
            B7 = consts[:, 12:13]
            S0 = consts[:, 13:14]; T0 = consts[:, 14:15]
            B8 = consts[0:NCLS, 15:16]

            w2_sb = wp.tile([128, 9 * 64], bf16, name="w2_sb", tag="w2ovl")
            nc.sync.dma_start(out=w2_sb[:], in_=w2_d[:])
            m1_sb = wp.tile([128, NSLOT * 128], bf16, name="m1_sb", tag="m1ovl")
            nc.sync.dma_start(out=m1_sb[:], in_=m1_d[:])
            w3_sb = wp.tile([128, NSLOT * 64], bf16, name="w3_sb", tag="w3ovl")
            nc.sync.dma_start(out=w3_sb[:], in_=w3_d[:])
            w4_sb = wp.tile([128, NSLOT * 64], bf16, name="w4_sb", tag="w4ovl")
            nc.sync.dma_start(out=w4_sb[:], in_=w4_d[:])

            ident = wp.tile([128, 128], f32, name="ident")
            from concourse.masks import make_identity
            make_identity(nc, ident[:])

            xT = wp.tile([128, NB], bf16, name="xT", tag="xTovl")
            F6 = wp.tile([128, 4 * NB], bf16, name="F6")
            A2 = big.tile([128, YS * XS * S], bf16, name="A2")
            A3 = big.tile([128, NSLOT * G], bf16, name="A3")

            f6v = F6[:].rearrange("q (m n) -> q m n", m=4, n=NB)
            a3v = A3[:].rearrange("q (sl g) -> q sl g", sl=NSLOT, g=G)

            # ---------------- zero A2 pad slots ----------------
            # regions (in slots): [0, 19), [35,37), [53,55), [71,73), [89,108)
            for lo, hi in ((0, XS + 1), (2 * XS - 1, 2 * XS + 1),
                           (3 * XS - 1, 3 * XS + 1), (4 * XS - 1, 4 * XS + 1),
                           (5 * XS - 1, YS * XS)):
                nc.gpsimd.memset(A2[:, lo * S:hi * S], 0.0)

            # ---------------- P0: load + transpose + bn0 ----------------
            with tc.tile_pool(name="p0ps", bufs=2, space="PSUM") as p0ps, \
                 tc.tile_pool(name="xstp", bufs=3) as xstp:
                ntile = NB // 128
                for t0 in range(0, ntile, 4):
                    ps0 = p0ps.tile([128, 512], f32, name="ps0", tag="ps0")
                    for i in range(4):
                        t = t0 + i
                        xst = xstp.tile([128, 128], f32, name="xst", tag="xst")
                        nc.sync.dma_start(out=xst[:], in_=x_d[t * 128:(t + 1) * 128, :])
                        nc.tensor.matmul(ps0[:, i * 128:(i + 1) * 128], xst[:], ident[:],
                                         is_transpose=True, start=True, stop=True,
                                         skip_group_check=True)
                    # xT = s0 * x^T + t0  (bn0; single channel so scalars)
                    nc.vector.tensor_scalar(xT[:, t0 * 128:(t0 + 4) * 128], ps0[:],
                                            S0, T0,
                                            mybir.AluOpType.mult, mybir.AluOpType.add)

            # tap list: (dy, dx), tap index t = (dy+1)*3 + (dx+1)
            TAPS = [(dy, dx) for dy in (-1, 0, 1) for dx in (-1, 0, 1)]

            for p in range(NPASS):
                po = p % 2
                g = p // 2
                xTp = xT[:, p * S:(p + 1) * S]

                # ---- conv1: dense K=128 M=128 (two parity pixels) ----
                with tc.tile_pool(name=f"c1ps{p}", bufs=2, space="PSUM") as c1pp:
                    for Y in range(YP):
                        for xh in range(W // 2):
                            ps = c1pp.tile([128, 2 * S], f32, name="psC1", tag="psC1")
                            for i in range(2):
                                s_idx = Y * W + 2 * xh + i
                                nc.tensor.matmul(
                                    ps[:, i * S:(i + 1) * S],
                                    m1_sb[:, s_idx * 128:(s_idx + 1) * 128],
                                    xTp, start=True, stop=True,
                                    skip_group_check=True)
                            evict(A2[:, a2c(Y, 2 * xh):a2c(Y, 2 * xh) + 2 * S],
                                  ps[:], B1)

                # ---- conv2: 9 column-shift taps, 4-chunk quadrant groups ----
                with tc.tile_pool(name=f"c2ps{p}", bufs=2, space="PSUM") as c2pp:
                    for cg in range(8):          # chunk-groups of 4 chunks
                        chunks = [divmod(4 * cg + i, 8) for i in range(4)]  # (Y, j2)
                        ps = [c2pp.tile([128, 2 * S], f32, name=f"psC2_{i}",
                                        tag=f"psC2_{i}") for i in range(4)]
                        for t, (dy, dx) in enumerate(TAPS):
                            wt = {r: w2_sb[r:r + 64, t * 64:(t + 1) * 64]
                                  for r in (0, 64)}
                            # per chunk: (even-out half, odd-out half)
                            mms = []
                            for i, (Y, j2) in enumerate(chunks):
                                role = (Y * 8 + j2) % 2  # 0=A(normal) 1=B(swap)
                                # even-out rows 2Y: input parity/ypair per dy
                                r_e = 0 if dy == 0 else 64
                                ys_e = -1 if dy == -1 else 0
                                # odd-out rows 2Y+1
                                r_o = 64 if dy == 0 else 0
                                ys_o = 1 if dy == 1 else 0
                                c_e = 64 * role
                                c_o = 64 - 64 * role
                                mms.append((i, r_e, c_e, Y + ys_e, j2, dx))
                                mms.append((i, r_o, c_o, Y + ys_o, j2, dx))
                            # issue order: cycle quadrants for concurrency
                            order = [0, 2, 1, 3, 4, 6, 5, 7]
                            for k in order:
                                i, r, cq, Yi, j2, dxi = mms[k]
                                base = a2c(Yi, 2 * j2 + dxi)
                                nc.tensor.matmul(
                                    ps[i][cq:cq + 64, :],
                                    wt[r],
                                    A2[r:r + 64, base:base + 2 * S],
                                    start=(t == 0), stop=(t == 8),
                                    tile_position=(r, cq),
                                    skip_group_check=True)
                        for i, (Y, j2) in enumerate(chunks):
                            s_idx = Y * W + 2 * j2
                            evict(a3v[:, s_idx:s_idx + 2, po * S:(po + 1) * S],
                                  ps[i][:].rearrange("q (u s) -> q u s", u=2, s=S),
                                  B2)

                if debug_taps and p == 0:
                    nc.sync.dma_start(out=dbg_a2_d[:], in_=A2[:])

                if po == 0:
                    continue

                # ======== P3: lc3 + lc4 + fc5 over group g ========
                with tc.tile_pool(name=f"f5ps{g}", bufs=1, space="PSUM") as f5pp, \
                     tc.tile_pool(name=f"lcps{g}", bufs=1, space="PSUM") as lcpp, \
                     tc.tile_pool(name=f"lcsb{g}", bufs=2) as lcsb, \
                     tc.tile_pool(name=f"wstp{g}", bufs=4) as wstp:
                    ps5 = [f5pp.tile([128, G], f32, name=f"ps5_{m}", tag=f"ps5_{m}")
                           for m in range(4)]
                    for kp in range(NSLOT // 2):
                        sA, sB = 2 * kp, 2 * kp + 1
                        wst = [wstp.tile([128, 512], bf16, name=f"wst{u}", tag=f"wst{u}")
                               for u in range(2)]
                        nc.sync.dma_start(out=wst[0][:], in_=fc5w_d[sA])
                        nc.sync.dma_start(out=wst[1][:], in_=fc5w_d[sB])
                        # lc3: 4 concurrent quadrant MMs
                        psA = lcpp.tile([128, G], f32, name="ps3a", tag="ps3a")
                        psB = lcpp.tile([128, G], f32, name="ps3b", tag="ps3b")
                        for r in (0, 64):
                            nc.tensor.matmul(
                                psA[r:r + 64, :],
                                w3_sb[r:r + 64, sA * 64:sA * 64 + 64],
                                a3v[r:r + 64, sA, :], start=True, stop=True,
                                tile_position=(r, r), skip_group_check=True)
                            nc.tensor.matmul(
                                psB[64 - r:128 - r, :],
                                w3_sb[r:r + 64, sB * 64:sB * 64 + 64],
                                a3v[r:r + 64, sB, :], start=True, stop=True,
                                tile_position=(r, 64 - r), skip_group_check=True)
                        tA = lcsb.tile([128, G], bf16, name="tA", tag="tA")
                        tB = lcsb.tile([128, G], bf16, name="tB", tag="tB")
                        evict(tA[:], psA[:], B3)
                        evict(tB[:], psB[:], B3)
                        # lc4
                        ps4A = lcpp.tile([128, G], f32, name="ps4a", tag="ps4a")
                        ps4B = lcpp.tile([128, G], f32, name="ps4b", tag="ps4b")
                        for r in (0, 64):
                            nc.tensor.matmul(
                                ps4A[r:r + 64, :],
                                w4_sb[r:r + 64, sA * 64:sA * 64 + 64],
                                tA[r:r + 64, :], start=True, stop=True,
                                tile_position=(r, r), skip_group_check=True)
                            nc.tensor.matmul(
                                ps4B[64 - r:128 - r, :],
                                w4_sb[r:r + 64, sB * 64:sB * 64 + 64],
                                tB[r:r + 64, :], start=True, stop=True,
                                tile_position=(r, 64 - r), skip_group_check=True)
                        FA = lcsb.tile([128, G], bf16, name="FA", tag="FA")
                        FB = lcsb.tile([128, G], bf16, name="FB", tag="FB")
                        evict(FA[:], ps4A[:], B4)
                        evict(FB[:], ps4B[:], B4)
                        # fc5 accumulation
                        for u, F in ((0, FA), (1, FB)):
                            sl = 2 * kp + u
                            for m in range(4):
                                nc.tensor.matmul(
                                    ps5[m][:, :], wst[u][:, 128 * m:128 * m + 128],
                                    F[:, :], start=(sl == 0), stop=(sl == NSLOT - 1))
                    for m in range(4):
                        evict(f6v[:, m, g * G:(g + 1) * G], ps5[m][:, :], B5[m])

            if debug_taps:
                nc.sync.dma_start(out=dbg_a3_d[:], in_=A3[:])
                nc.sync.dma_start(out=dbg_f6_d[:], in_=F6[:])

            # ============ fc6 / fc7 / fc8 ============
            fc6w_sb = wp.tile([128, 16 * 128], bf16, name="fc6w_sb", tag="w3ovl")
            nc.sync.dma_start(out=fc6w_sb[:], in_=fc6w_d[:])
            fc78w_sb = wp.tile([128, 4 * 128 + NCLS], bf16, name="fc78w_sb", tag="xTovl")
            nc.sync.dma_start(out=fc78w_sb[:], in_=fc78w_d[:])
            F7 = wp.tile([128, 4 * NB], bf16, name="F7", tag="m1ovl")
            F8 = wp.tile([128, NB], bf16, name="F8", tag="w2ovl")
            y_sb = wp.tile([NCLS, NB], f32, name="y_sb", tag="w4ovl")
            f7v = F7[:].rearrange("q (m n) -> q m n", m=4, n=NB)

            with tc.tile_pool(name="fcps", bufs=4, space="PSUM") as fcpp, \
                 tc.tile_pool(name="fc8ps", bufs=2, space="PSUM") as fc8pp:
                for n in range(NSPL):
                    n0 = n * NCOLS
                    for m in range(4):
                        ps6 = fcpp.tile([128, NCOLS], f32, name="ps6", tag="ps6")
                        for jj in range(4):
                            nc.tensor.matmul(ps6[:, :],
                                             fc6w_sb[:, (4 * jj + m) * 128:(4 * jj + m) * 128 + 128],
                                             f6v[:, jj, n0:n0 + NCOLS],
                                             start=(jj == 0), stop=(jj == 3))
                        evict(f7v[:, m, n0:n0 + NCOLS], ps6[:, :], B6[m])
                for n in range(NSPL):
                    n0 = n * NCOLS
                    ps7 = fcpp.tile([128, NCOLS], f32, name="ps7", tag="ps6")
                    for jj in range(4):
                        nc.tensor.matmul(ps7[:, :],
                                         fc78w_sb[:, 128 * jj:128 * jj + 128],
                                         f7v[:, jj, n0:n0 + NCOLS],
                                         start=(jj == 0), stop=(jj == 3))
                    evict(F8[:, n0:n0 + NCOLS], ps7[:, :], B7)
                for n in range(NSPL):
                    n0 = n * NCOLS
                    ps8 = fc8pp.tile([NCLS, NCOLS], f32, name="ps8", tag="ps8")
                    nc.tensor.matmul(ps8[:, :], fc78w_sb[:, 512:512 + NCLS],
                                     F8[:, n0:n0 + NCOLS], start=True, stop=True)
                    nc.vector.tensor_scalar(y_sb[:, n0:n0 + NCOLS], ps8[:, :], B8, None,
                                            mybir.AluOpType.add)
            nc.sync.dma_start(out=y_d[:], in_=y_sb[:])

    nc.compile()
    return nc


# ---------------------------------------------------------------------------
# host-side weight preparation
# ---------------------------------------------------------------------------

def _bn_affine(p):
    g, b, m, v = p[0], p[1], p[2], p[3]
    s = g / np.sqrt(v + EPS)
    return s.astype(np.float32), (b - m * s).astype(np.float32)


def prep_weights(inputs):
    bf = ml_dtypes.bfloat16
    s0, t0 = _bn_affine(inputs['bn0']); s0, t0 = float(s0[0]), float(t0[0])
    s1, t1 = _bn_affine(inputs['bn1'])
    s2, t2 = _bn_affine(inputs['bn2'])
    s3, t3 = _bn_affine(inputs['bn3'])
    s4, t4 = _bn_affine(inputs['bn4'])
    s5, t5 = _bn_affine(inputs['bn5']); s5, t5 = float(s5[0]), float(t5[0])
    s6, t6 = _bn_affine(inputs['bn6']); s6, t6 = float(s6[0]), float(t6[0])
    s7, t7 = _bn_affine(inputs['bn7']); s7, t7 = float(s7[0]), float(t7[0])

    # conv1 dense: m1[praw, s_idx*128 + i*64 + o] for pixel (2Y+i, x)
    w1 = np.asarray(inputs['conv1_w'], np.float32)      # [64,1,3,3]
    m1 = np.zeros((128, NSLOT * 128), np.float32)
    for s_idx in range(NSLOT):
        Y, x = divmod(s_idx, W)
        for i in range(2):
            py = 2 * Y + i
            for ky in range(3):
                for kx in range(3):
                    iy, jx = py + ky - 1, x + kx - 1
                    if 0 <= iy < H and 0 <= jx < W:
                        praw = 8 * jx + iy
                        m1[praw, s_idx * 128 + i * 64:s_idx * 128 + i * 64 + 64] += \
                            s1 * w1[:, 0, ky, kx]
    bias1 = (s1 * np.asarray(inputs['conv1_b'], np.float32) + t1)

    # conv2 taps: [cin, cout] blocks duplicated on both partition halves
    w2r = np.asarray(inputs['conv2_w'], np.float32)     # [64,64,3,3]
    w2 = np.zeros((128, 9 * 64), np.float32)
    for t in range(9):
        ky, kx = t // 3, t % 3
        blk = (s2[None, :] * w2r[:, :, ky, kx].T)       # [cin, cout]
        w2[0:64, 64 * t:64 * t + 64] = blk
        w2[64:128, 64 * t:64 * t + 64] = blk
    bias2 = (s2 * np.asarray(inputs['conv2_b'], np.float32) + t2)

    # lc weights: per slot one 64-col block; rows 0:64 = quadrant at r=0,
    # rows 64:128 = quadrant at r=64. A-position slots (even s_idx) keep
    # layout; B-position slots swap output halves.
    def lc_pack(wname, s, sin_fn):
        wr = np.asarray(inputs[wname], np.float32)      # [o, c, h, w]
        out = np.zeros((128, NSLOT * 64), np.float32)
        for s_idx in range(NSLOT):
            Y, x = divmod(s_idx, W)
            sin = sin_fn(s_idx)
            for r in (0, 64):
                u = sin ^ (1 if r == 64 else 0)
                py = 2 * Y + u
                blk = s[None, :] * wr[:, :, py, x].T    # [cin, cout]
                out[r:r + 64, s_idx * 64:s_idx * 64 + 64] = blk
        return out
    w3 = lc_pack('lc3_w', s3, lambda s_idx: sig1(*divmod(s_idx, W)))
    w4 = lc_pack('lc4_w', s4, lambda s_idx: sig1(*divmod(s_idx, W)) ^ (s_idx % 2))

    # fc5: K-chunk per slot; row r -> ch r%64, parity (r>=64)^sig5, sig5=sig1
    fc5 = np.asarray(inputs['fc5_w'], np.float32)       # [512, 8192]
    fc5w = np.zeros((NSLOT, 128, 512), np.float32)
    for s_idx in range(NSLOT):
        Y, x = divmod(s_idx, W)
        sg = sig1(Y, x)
        ch = np.arange(128) % 64
        u = (np.arange(128) >= 64).astype(np.int64) ^ sg
        flat = ch * 128 + (2 * Y + u) * W + x
        fc5w[s_idx] = s5 * fc5[:, flat].T
    bias5 = s5 * np.asarray(inputs['fc5_b'], np.float32) + t5   # [512]

    fc6 = np.asarray(inputs['fc6_w'], np.float32)       # [512, 512]
    fc6w = np.zeros((128, 16 * 128), np.float32)
    for jj in range(4):
        for m in range(4):
            blk = s6 * fc6[128 * m:128 * m + 128, 128 * jj:128 * jj + 128].T
            fc6w[:, (4 * jj + m) * 128:(4 * jj + m) * 128 + 128] = blk
    bias6 = s6 * np.asarray(inputs['fc6_b'], np.float32) + t6   # [512]

    fc7 = np.asarray(inputs['fc7_w'], np.float32)       # [128, 512]
    fc78w = np.zeros((128, 4 * 128 + NCLS), np.float32)
    for jj in range(4):
        fc78w[:, 128 * jj:128 * jj + 128] = s7 * fc7[:, 128 * jj:128 * jj + 128].T
    bias7 = s7 * np.asarray(inputs['fc7_b'], np.float32) + t7   # [128]
    fc8 = np.asarray(inputs['fc8_w'], np.float32)       # [8, 128]
    fc78w[:, 512:512 + NCLS] = fc8.T
    bias8 = np.asarray(inputs['fc8_b'], np.float32)     # [8]

    consts = np.zeros((128, 16), np.float32)
    consts[:, 0] = np.concatenate([bias1, bias1])
    consts[:, 1] = np.concatenate([bias2, bias2])
    consts[:, 2] = np.concatenate([t3, t3])
    consts[:, 3] = np.concatenate([t4, t4])
    for m in range(4):
        consts[:, 4 + m] = bias5[128 * m:128 * m + 128]
        consts[:, 8 + m] = bias6[128 * m:128 * m + 128]
    consts[:, 12] = bias7
    consts[:, 13] = s0
    consts[:, 14] = t0
    consts[0:NCLS, 15] = bias8

    return {
        'm1': m1.astype(bf), 'w2': w2.astype(bf), 'w3': w3.astype(bf),
        'w4': w4.astype(bf), 'fc5w': fc5w.astype(bf), 'fc6w': fc6w.astype(bf),
        'fc78w': fc78w.astype(bf), 'consts': consts,
    }


_cache = {}


def _get_nc(NB=1024, S=256, debug_taps=False):
    key = (NB, S, debug_taps)
    if key not in _cache:
        _cache[key] = build(NB, S, debug_taps)
    return _cache[key]


def kernel(**inputs):
    from concourse.bass_utils import run_bass_kernel_spmd
    x = np.asarray(inputs['x'], np.float32)
    B = x.shape[0]
    NB = B // NCORES
    xf = x.reshape(B, 128)
    w = prep_weights(inputs)
    nc = _get_nc(NB=NB, S=256)
    in_maps = []
    for c in range(NCORES):
        m = dict(w)
        m['x'] = np.ascontiguousarray(xf[c * NB:(c + 1) * NB])
        in_maps.append(m)
    res = run_bass_kernel_spmd(nc, in_maps, list(range(NCORES)))
    return _assemble(res, B, NB)


def _assemble(res, B, NB):
    out = np.empty((B, NCLS), np.float32)
    for c in range(NCORES):
        yc = np.asarray(res.results[c]['y'], np.float32)   # [8, NB]
        out[c * NB:(c + 1) * NB] = yc.T
    return out


def run_traced(inputs, tmpdir=None):
    """Like kernel() but with NTFF tracing; returns (out, BassKernelResults)."""
    from concourse.bass_utils import run_bass_kernel_spmd
    x = np.asarray(inputs['x'], np.float32)
    B = x.shape[0]
    NB = B // NCORES
    xf = x.reshape(B, 128)
    w = prep_weights(inputs)
    nc = _get_nc(NB=NB, S=256)
    in_maps = []
    for c in range(NCORES):
        m = dict(w)
        m['x'] = np.ascontiguousarray(xf[c * NB:(c + 1) * NB])
        in_maps.append(m)
    res = run_bass_kernel_spmd(nc, in_maps, list(range(NCORES)), trace=True,
                               tmpdir=tmpdir)
    return _assemble(res, B, NB), res


# revision 5
# speedup vs baseline: 4.0789x; 1.2689x over previous
"""Trainium2 Bass kernel for nn_CapgMyoNet (dense CNN), 8-core data-parallel.

Network (per sample): permute(8,16) -> bn0 -> conv3x3(1->64)+bn+relu
  -> conv3x3(64->64)+bn+relu -> 2x locally-connected 1x1 (per-pixel 64x64)
  -> fc 8192->512 -> fc 512->512 -> fc 512->128 -> fc 128->8
All bn folded into weights/biases on host. bf16 matmuls, fp32 accumulate.

Layout: activations live as [128 partitions = 64ch x row-parity], columns =
(ypair, x, sample) over a zero-padded slot grid (6 ypairs x 18 x incl pads).
- conv1: K=128 raw-pixel dense matmul, M=128 = two row-parity pixels packed.
- conv2: 9 taps = pure column-shift matmuls accumulated in PSUM; chunks
  alternate normal/parity-swapped output column groups so 4 independent
  64x64 quadrant matmuls run concurrently (full PE array).
- lc3/lc4: per-slot 64x64 quadrant matmuls, slot pairs pack 4 quadrants.
- fc5: K-chunk = one slot's 128 partitions; 4 M-chunks accumulate per group.
- fc6/7/8: straightforward K-chunked matmuls, N=512.
All PSUM->SBUF evictions are contiguous [128,512] with bias+relu fused.
"""
import numpy as np
import ml_dtypes

import concourse.bass as bass
import concourse.bacc as bacc
import concourse.mybir as mybir
import concourse.tile as tile

bf16 = mybir.dt.bfloat16
f32 = mybir.dt.float32

H, W, C, NCLS = 8, 16, 64, 8
EPS = 1e-5
NCORES = 8

YP = 4       # real ypair rows
XS = 18      # x slots incl 1 pad each side
YS = 6       # ypair slots incl 1 pad each side
NSLOT = YP * W  # 64 real slots


def sig1(Y, x):
    """Parity-swap flag of A3 slot (Y, x): conv2 chunk role."""
    return (Y * 8 + x // 2) % 2


def build(NB=1024, S=256, debug_taps=False):
    """Per-core program. NB samples/core, S samples per conv pass."""
    NPASS = NB // S
    G = 2 * S           # lc/fc5 group size
    NG = NB // G
    NSPL = max(1, NB // 512)
    NCOLS = NB // NSPL

    nc = bacc.Bacc("TRN2", target_bir_lowering=False, debug=False)
    ev_ct = [0]

    def evict(out_ap, in_ap, bias_ap, relu=True):
        """Alternating-engine psum->sbuf eviction with bias (+relu)."""
        ev_ct[0] += 1
        if ev_ct[0] % 2 == 0:
            if relu:
                nc.scalar.activation(out_ap, in_ap,
                                     mybir.ActivationFunctionType.Relu,
                                     bias=bias_ap)
            else:
                nc.vector.tensor_scalar(out_ap, in_ap, bias_ap, None,
                                        mybir.AluOpType.add)
        else:
            if relu:
                nc.vector.tensor_scalar(out_ap, in_ap, bias_ap, 0.0,
                                        mybir.AluOpType.add,
                                        mybir.AluOpType.max)
            else:
                nc.vector.tensor_scalar(out_ap, in_ap, bias_ap, None,
                                        mybir.AluOpType.add)

    def a2c(Y, x):
        """A2 column base of slot (ypair Y, x)."""
        return ((Y + 1) * XS + (x + 1)) * S

    with tile.TileContext(nc) as tc:
        from contextlib import ExitStack
        es = ExitStack()
        with es:
            dram = es.enter_context(tc.tile_pool(name="dram", bufs=1, space="DRAM"))
            wp = es.enter_context(tc.tile_pool(name="wp", bufs=1))
            big = es.enter_context(tc.tile_pool(name="big", bufs=1))

            # ---------------- DRAM I/O ----------------
            x_d = dram.tile([NB, 128], f32, kind="ExternalInput", name="x", uniquify=False)
            m1_d = dram.tile([128, NSLOT * 128], bf16, kind="ExternalInput", name="m1", uniquify=False)
            w2_d = dram.tile([128, 9 * 64], bf16, kind="ExternalInput", name="w2", uniquify=False)
            w3_d = dram.tile([128, NSLOT * 64], bf16, kind="ExternalInput", name="w3", uniquify=False)
            w4_d = dram.tile([128, NSLOT * 64], bf16, kind="ExternalInput", name="w4", uniquify=False)
            fc5w_d = dram.tile([NSLOT, 128, 512], bf16, kind="ExternalInput", name="fc5w", uniquify=False)
            fc6w_d = dram.tile([128, 16 * 128], bf16, kind="ExternalInput", name="fc6w", uniquify=False)
            fc78w_d = dram.tile([128, 4 * 128 + NCLS], bf16, kind="ExternalInput", name="fc78w", uniquify=False)
            consts_d = dram.tile([128, 16], f32, kind="ExternalInput", name="consts", uniquify=False)
            y_d = dram.tile([NCLS, NB], f32, kind="ExternalOutput", name="y", uniquify=False)
            if debug_taps:
                dbg_a2_d = dram.tile([128, YS * XS * S], bf16, kind="ExternalOutput", name="dbg_a2", uniquify=False)
                dbg_a3_d = dram.tile([128, NSLOT * G], bf16, kind="ExternalOutput", name="dbg_a3", uniquify=False)
                dbg_f6_d = dram.tile([128, 4 * NB], bf16, kind="ExternalOutput", name="dbg_f6", uniquify=False)

            # ---------------- persistent SBUF ----------------
            consts = wp.tile([128, 16], f32, name="consts_sb")
            nc.sync.dma_start(out=consts[:], in_=consts_d[:])
            B1 = consts[:, 0:1]; B2 = consts[:, 1:2]
            B3 = consts[:, 2:3]; B4 = consts[:, 3:4]
            B5 = [consts[:, 4 + m:5 + m] for m in range(4)]
            B6 = [consts[:, 8 + m:9 + m] for m in range(4)]
            B7 = consts[:, 12:13]
            S0 = consts[:, 13:14]; T0 = consts[:, 14:15]
            B8 = consts[0:NCLS, 15:16]

            ident = wp.tile([128, 128], f32, name="ident")
            from concourse.masks import make_identity
            make_identity(nc, ident[:])

            xT = wp.tile([128, NB], bf16, name="xT", tag="xTovl")
            F6 = wp.tile([128, 4 * NB], bf16, name="F6")
            A2 = big.tile([128, YS * XS * S], bf16, name="A2")
            A3 = big.tile([128, NSLOT * G], bf16, name="A3")

            f6v = F6[:].rearrange("q (m n) -> q m n", m=4, n=NB)
            a3v = A3[:].rearrange("q (sl g) -> q sl g", sl=NSLOT, g=G)

            # ---------------- zero A2 pad slots ----------------
            # regions (in slots): [0, 19), [35,37), [53,55), [71,73), [89,108)
            for lo, hi in ((0, XS + 1), (2 * XS - 1, 2 * XS + 1),
                           (3 * XS - 1, 3 * XS + 1), (4 * XS - 1, 4 * XS + 1),
                           (5 * XS - 1, YS * XS)):
                nc.gpsimd.memset(A2[:, lo * S:hi * S], 0.0)

            # ---------------- P0: load + transpose + bn0 ----------------
            # (emitted before the weight DMAs so the x tiles are first in
            # the DMA queue and the PE starts immediately)
            with tc.tile_pool(name="p0ps", bufs=2, space="PSUM") as p0ps, \
                 tc.tile_pool(name="xstp", bufs=3) as xstp:
                ntile = NB // 128
                for t0 in range(0, ntile, 4):
                    ps0 = p0ps.tile([128, 512], f32, name="ps0", tag="ps0")
                    for i in range(4):
                        t = t0 + i
                        xst = xstp.tile([128, 128], f32, name="xst", tag="xst")
                        nc.sync.dma_start(out=xst[:], in_=x_d[t * 128:(t + 1) * 128, :])
                        nc.tensor.matmul(ps0[:, i * 128:(i + 1) * 128], xst[:], ident[:],
                                         is_transpose=True, start=True, stop=True,
                                         skip_group_check=True)
                    # xT = s0 * x^T + t0  (bn0; single channel so scalars)
                    nc.vector.tensor_scalar(xT[:, t0 * 128:(t0 + 4) * 128], ps0[:],
                                            S0, T0,
                                            mybir.AluOpType.mult, mybir.AluOpType.add)

            m1_sb = wp.tile([128, NSLOT * 128], bf16, name="m1_sb", tag="m1ovl")
            nc.sync.dma_start(out=m1_sb[:], in_=m1_d[:])
            w2_sb = wp.tile([128, 9 * 64], bf16, name="w2_sb", tag="w2ovl")
            nc.sync.dma_start(out=w2_sb[:], in_=w2_d[:])
            w3_sb = wp.tile([128, NSLOT * 64], bf16, name="w3_sb", tag="w3ovl")
            nc.sync.dma_start(out=w3_sb[:], in_=w3_d[:])
            w4_sb = wp.tile([128, NSLOT * 64], bf16, name="w4_sb", tag="w4ovl")
            nc.sync.dma_start(out=w4_sb[:], in_=w4_d[:])

            # tap list: (dy, dx), tap index t = (dy+1)*3 + (dx+1)
            TAPS = [(dy, dx) for dy in (-1, 0, 1) for dx in (-1, 0, 1)]

            for p in range(NPASS):
                po = p % 2
                g = p // 2
                xTp = xT[:, p * S:(p + 1) * S]

                # ---- conv1: dense K=128 M=128 (two parity pixels) ----
                with tc.tile_pool(name=f"c1ps{p}", bufs=4, space="PSUM") as c1pp:
                    for Y in range(YP):
                        for xh in range(W // 2):
                            ps = c1pp.tile([128, 2 * S], f32, name="psC1", tag="psC1")
                            for i in range(2):
                                s_idx = Y * W + 2 * xh + i
                                nc.tensor.matmul(
                                    ps[:, i * S:(i + 1) * S],
                                    m1_sb[:, s_idx * 128:(s_idx + 1) * 128],
                                    xTp, start=True, stop=True,
                                    skip_group_check=True)
                            evict(A2[:, a2c(Y, 2 * xh):a2c(Y, 2 * xh) + 2 * S],
                                  ps[:], B1)

                # ---- conv2: 9 column-shift taps, 4-chunk quadrant groups ----
                with tc.tile_pool(name=f"c2ps{p}", bufs=2, space="PSUM") as c2pp:
                    for cg in range(8):          # chunk-groups of 4 chunks
                        chunks = [divmod(4 * cg + i, 8) for i in range(4)]  # (Y, j2)
                        ps = [c2pp.tile([128, 2 * S], f32, name=f"psC2_{i}",
                                        tag=f"psC2_{i}") for i in range(4)]
                        for t, (dy, dx) in enumerate(TAPS):
                            wt = {r: w2_sb[r:r + 64, t * 64:(t + 1) * 64]
                                  for r in (0, 64)}
                            # per chunk: (even-out half, odd-out half)
                            mms = []
                            for i, (Y, j2) in enumerate(chunks):
                                role = (Y * 8 + j2) % 2  # 0=A(normal) 1=B(swap)
                                # even-out rows 2Y: input parity/ypair per dy
                                r_e = 0 if dy == 0 else 64
                                ys_e = -1 if dy == -1 else 0
                                # odd-out rows 2Y+1
                                r_o = 64 if dy == 0 else 0
                                ys_o = 1 if dy == 1 else 0
                                c_e = 64 * role
                                c_o = 64 - 64 * role
                                mms.append((i, r_e, c_e, Y + ys_e, j2, dx))
                                mms.append((i, r_o, c_o, Y + ys_o, j2, dx))
                            # issue order: cycle quadrants for concurrency
                            order = [0, 2, 1, 3, 4, 6, 5, 7]
                            for k in order:
                                i, r, cq, Yi, j2, dxi = mms[k]
                                base = a2c(Yi, 2 * j2 + dxi)
                                nc.tensor.matmul(
                                    ps[i][cq:cq + 64, :],
                                    wt[r],
                                    A2[r:r + 64, base:base + 2 * S],
                                    start=(t == 0), stop=(t == 8),
                                    tile_position=(r, cq),
                                    skip_group_check=True)
                        for i, (Y, j2) in enumerate(chunks):
                            s_idx = Y * W + 2 * j2
                            evict(a3v[:, s_idx:s_idx + 2, po * S:(po + 1) * S],
                                  ps[i][:].rearrange("q (u s) -> q u s", u=2, s=S),
                                  B2)

                if debug_taps and p == 0:
                    nc.sync.dma_start(out=dbg_a2_d[:], in_=A2[:])

                if po == 0:
                    continue

                # ======== P3: lc3 + lc4 + fc5 over group g ========
                # 2-stage software pipeline: step k runs lc3(k), lc4(k-1),
                # fc5(k-2) so every PE instruction's inputs were evicted a
                # full step earlier and the PE never waits on DVE/ACT.
                with tc.tile_pool(name=f"f5ps{g}", bufs=1, space="PSUM") as f5pp, \
                     tc.tile_pool(name=f"lcps{g}", bufs=1, space="PSUM") as lcpp, \
                     tc.tile_pool(name=f"lcsb{g}", bufs=2) as lcsb, \
                     tc.tile_pool(name=f"wstp{g}", bufs=3) as wstp:
                    ps5 = [f5pp.tile([128, G], f32, name=f"ps5_{m}", tag=f"ps5_{m}")
                           for m in range(4)]
                    NKP = NSLOT // 2
                    tT, FT, WT = {}, {}, {}
                    for step in range(NKP + 2):
                        if step < NKP:
                            kp = step
                            sA, sB = 2 * kp, 2 * kp + 1
                            wst = [wstp.tile([128, 512], bf16, name=f"wst{u}",
                                             tag=f"wst{u}") for u in range(2)]
                            nc.sync.dma_start(out=wst[0][:], in_=fc5w_d[sA])
                            nc.sync.dma_start(out=wst[1][:], in_=fc5w_d[sB])
                            WT[kp] = wst
                            # lc3: 4 concurrent quadrant MMs
                            psA = lcpp.tile([128, G], f32, name="ps3a", tag="ps3a")
                            psB = lcpp.tile([128, G], f32, name="ps3b", tag="ps3b")
                            for r in (0, 64):
                                nc.tensor.matmul(
                                    psA[r:r + 64, :],
                                    w3_sb[r:r + 64, sA * 64:sA * 64 + 64],
                                    a3v[r:r + 64, sA, :], start=True, stop=True,
                                    tile_position=(r, r), skip_group_check=True)
                                nc.tensor.matmul(
                                    psB[64 - r:128 - r, :],
                                    w3_sb[r:r + 64, sB * 64:sB * 64 + 64],
                                    a3v[r:r + 64, sB, :], start=True, stop=True,
                                    tile_position=(r, 64 - r), skip_group_check=True)
                            tA = lcsb.tile([128, G], bf16, name="tA", tag="tA")
                            tB = lcsb.tile([128, G], bf16, name="tB", tag="tB")
                            evict(tA[:], psA[:], B3)
                            evict(tB[:], psB[:], B3)
                            tT[kp] = (tA, tB)
                        if 1 <= step <= NKP:
                            kq = step - 1
                            sA, sB = 2 * kq, 2 * kq + 1
                            tA, tB = tT.pop(kq)
                            ps4A = lcpp.tile([128, G], f32, name="ps4a", tag="ps4a")
                            ps4B = lcpp.tile([128, G], f32, name="ps4b", tag="ps4b")
                            for r in (0, 64):
                                nc.tensor.matmul(
                                    ps4A[r:r + 64, :],
                                    w4_sb[r:r + 64, sA * 64:sA * 64 + 64],
                                    tA[r:r + 64, :], start=True, stop=True,
                                    tile_position=(r, r), skip_group_check=True)
                                nc.tensor.matmul(
                                    ps4B[64 - r:128 - r, :],
                                    w4_sb[r:r + 64, sB * 64:sB * 64 + 64],
                                    tB[r:r + 64, :], start=True, stop=True,
                                    tile_position=(r, 64 - r), skip_group_check=True)
                            FA = lcsb.tile([128, G], bf16, name="FA", tag="FA")
                            FB = lcsb.tile([128, G], bf16, name="FB", tag="FB")
                            evict(FA[:], ps4A[:], B4)
                            evict(FB[:], ps4B[:], B4)
                            FT[kq] = (FA, FB)
                        if step >= 2:
                            kr = step - 2
                            wst = WT.pop(kr)
                            for u, F in zip((0, 1), FT.pop(kr)):
                                sl = 2 * kr + u
                                for m in range(4):
                                    nc.tensor.matmul(
                                        ps5[m][:, :],
                                        wst[u][:, 128 * m:128 * m + 128],
                                        F[:, :], start=(sl == 0),
                                        stop=(sl == NSLOT - 1))
                    for m in range(4):
                        evict(f6v[:, m, g * G:(g + 1) * G], ps5[m][:, :], B5[m])

            if debug_taps:
                nc.sync.dma_start(out=dbg_a3_d[:], in_=A3[:])
                nc.sync.dma_start(out=dbg_f6_d[:], in_=F6[:])

            # ============ fc6 / fc7 / fc8 ============
            fc6w_sb = wp.tile([128, 16 * 128], bf16, name="fc6w_sb", tag="w3ovl")
            nc.sync.dma_start(out=fc6w_sb[:], in_=fc6w_d[:])
            fc78w_sb = wp.tile([128, 4 * 128 + NCLS], bf16, name="fc78w_sb", tag="xTovl")
            nc.sync.dma_start(out=fc78w_sb[:], in_=fc78w_d[:])
            F7 = wp.tile([128, 4 * NB], bf16, name="F7", tag="m1ovl")
            F8 = wp.tile([128, NB], bf16, name="F8", tag="w2ovl")
            y_sb = wp.tile([NCLS, NB], f32, name="y_sb", tag="w4ovl")
            f7v = F7[:].rearrange("q (m n) -> q m n", m=4, n=NB)

            with tc.tile_pool(name="fcps", bufs=4, space="PSUM") as fcpp, \
                 tc.tile_pool(name="fc8ps", bufs=2, space="PSUM") as fc8pp:
                for n in range(NSPL):
                    n0 = n * NCOLS
                    for m in range(4):
                        ps6 = fcpp.tile([128, NCOLS], f32, name="ps6", tag="ps6")
                        for jj in range(4):
                            nc.tensor.matmul(ps6[:, :],
                                             fc6w_sb[:, (4 * jj + m) * 128:(4 * jj + m) * 128 + 128],
                                             f6v[:, jj, n0:n0 + NCOLS],
                                             start=(jj == 0), stop=(jj == 3))
                        evict(f7v[:, m, n0:n0 + NCOLS], ps6[:, :], B6[m])
                for n in range(NSPL):
                    n0 = n * NCOLS
                    ps7 = fcpp.tile([128, NCOLS], f32, name="ps7", tag="ps6")
                    for jj in range(4):
                        nc.tensor.matmul(ps7[:, :],
                                         fc78w_sb[:, 128 * jj:128 * jj + 128],
                                         f7v[:, jj, n0:n0 + NCOLS],
                                         start=(jj == 0), stop=(jj == 3))
                    evict(F8[:, n0:n0 + NCOLS], ps7[:, :], B7)
                for n in range(NSPL):
                    n0 = n * NCOLS
                    ps8 = fc8pp.tile([NCLS, NCOLS], f32, name="ps8", tag="ps8")
                    nc.tensor.matmul(ps8[:, :], fc78w_sb[:, 512:512 + NCLS],
                                     F8[:, n0:n0 + NCOLS], start=True, stop=True)
                    nc.vector.tensor_scalar(y_sb[:, n0:n0 + NCOLS], ps8[:, :], B8, None,
                                            mybir.AluOpType.add)
            nc.sync.dma_start(out=y_d[:], in_=y_sb[:])

    nc.compile()
    return nc


# ---------------------------------------------------------------------------
# host-side weight preparation
# ---------------------------------------------------------------------------

def _bn_affine(p):
    g, b, m, v = p[0], p[1], p[2], p[3]
    s = g / np.sqrt(v + EPS)
    return s.astype(np.float32), (b - m * s).astype(np.float32)


def prep_weights(inputs):
    bf = ml_dtypes.bfloat16
    s0, t0 = _bn_affine(inputs['bn0']); s0, t0 = float(s0[0]), float(t0[0])
    s1, t1 = _bn_affine(inputs['bn1'])
    s2, t2 = _bn_affine(inputs['bn2'])
    s3, t3 = _bn_affine(inputs['bn3'])
    s4, t4 = _bn_affine(inputs['bn4'])
    s5, t5 = _bn_affine(inputs['bn5']); s5, t5 = float(s5[0]), float(t5[0])
    s6, t6 = _bn_affine(inputs['bn6']); s6, t6 = float(s6[0]), float(t6[0])
    s7, t7 = _bn_affine(inputs['bn7']); s7, t7 = float(s7[0]), float(t7[0])

    # conv1 dense: m1[praw, s_idx*128 + i*64 + o] for pixel (2Y+i, x)
    w1 = np.asarray(inputs['conv1_w'], np.float32)      # [64,1,3,3]
    m1 = np.zeros((128, NSLOT * 128), np.float32)
    for s_idx in range(NSLOT):
        Y, x = divmod(s_idx, W)
        for i in range(2):
            py = 2 * Y + i
            for ky in range(3):
                for kx in range(3):
                    iy, jx = py + ky - 1, x + kx - 1
                    if 0 <= iy < H and 0 <= jx < W:
                        praw = 8 * jx + iy
                        m1[praw, s_idx * 128 + i * 64:s_idx * 128 + i * 64 + 64] += \
                            s1 * w1[:, 0, ky, kx]
    bias1 = (s1 * np.asarray(inputs['conv1_b'], np.float32) + t1)

    # conv2 taps: [cin, cout] blocks duplicated on both partition halves
    w2r = np.asarray(inputs['conv2_w'], np.float32)     # [64,64,3,3]
    w2 = np.zeros((128, 9 * 64), np.float32)
    for t in range(9):
        ky, kx = t // 3, t % 3
        blk = (s2[None, :] * w2r[:, :, ky, kx].T)       # [cin, cout]
        w2[0:64, 64 * t:64 * t + 64] = blk
        w2[64:128, 64 * t:64 * t + 64] = blk
    bias2 = (s2 * np.asarray(inputs['conv2_b'], np.float32) + t2)

    # lc weights: per slot one 64-col block; rows 0:64 = quadrant at r=0,
    # rows 64:128 = quadrant at r=64. A-position slots (even s_idx) keep
    # layout; B-position slots swap output halves.
    def lc_pack(wname, s, sin_fn):
        wr = np.asarray(inputs[wname], np.float32)      # [o, c, h, w]
        out = np.zeros((128, NSLOT * 64), np.float32)
        for s_idx in range(NSLOT):
            Y, x = divmod(s_idx, W)
            sin = sin_fn(s_idx)
            for r in (0, 64):
                u = sin ^ (1 if r == 64 else 0)
                py = 2 * Y + u
                blk = s[None, :] * wr[:, :, py, x].T    # [cin, cout]
                out[r:r + 64, s_idx * 64:s_idx * 64 + 64] = blk
        return out
    w3 = lc_pack('lc3_w', s3, lambda s_idx: sig1(*divmod(s_idx, W)))
    w4 = lc_pack('lc4_w', s4, lambda s_idx: sig1(*divmod(s_idx, W)) ^ (s_idx % 2))

    # fc5: K-chunk per slot; row r -> ch r%64, parity (r>=64)^sig5, sig5=sig1
    fc5 = np.asarray(inputs['fc5_w'], np.float32)       # [512, 8192]
    fc5w = np.zeros((NSLOT, 128, 512), np.float32)
    for s_idx in range(NSLOT):
        Y, x = divmod(s_idx, W)
        sg = sig1(Y, x)
        ch = np.arange(128) % 64
        u = (np.arange(128) >= 64).astype(np.int64) ^ sg
        flat = ch * 128 + (2 * Y + u) * W + x
        fc5w[s_idx] = s5 * fc5[:, flat].T
    bias5 = s5 * np.asarray(inputs['fc5_b'], np.float32) + t5   # [512]

    fc6 = np.asarray(inputs['fc6_w'], np.float32)       # [512, 512]
    fc6w = np.zeros((128, 16 * 128), np.float32)
    for jj in range(4):
        for m in range(4):
            blk = s6 * fc6[128 * m:128 * m + 128, 128 * jj:128 * jj + 128].T
            fc6w[:, (4 * jj + m) * 128:(4 * jj + m) * 128 + 128] = blk
    bias6 = s6 * np.asarray(inputs['fc6_b'], np.float32) + t6   # [512]

    fc7 = np.asarray(inputs['fc7_w'], np.float32)       # [128, 512]
    fc78w = np.zeros((128, 4 * 128 + NCLS), np.float32)
    for jj in range(4):
        fc78w[:, 128 * jj:128 * jj + 128] = s7 * fc7[:, 128 * jj:128 * jj + 128].T
    bias7 = s7 * np.asarray(inputs['fc7_b'], np.float32) + t7   # [128]
    fc8 = np.asarray(inputs['fc8_w'], np.float32)       # [8, 128]
    fc78w[:, 512:512 + NCLS] = fc8.T
    bias8 = np.asarray(inputs['fc8_b'], np.float32)     # [8]

    consts = np.zeros((128, 16), np.float32)
    consts[:, 0] = np.concatenate([bias1, bias1])
    consts[:, 1] = np.concatenate([bias2, bias2])
    consts[:, 2] = np.concatenate([t3, t3])
    consts[:, 3] = np.concatenate([t4, t4])
    for m in range(4):
        consts[:, 4 + m] = bias5[128 * m:128 * m + 128]
        consts[:, 8 + m] = bias6[128 * m:128 * m + 128]
    consts[:, 12] = bias7
    consts[:, 13] = s0
    consts[:, 14] = t0
    consts[0:NCLS, 15] = bias8

    return {
        'm1': m1.astype(bf), 'w2': w2.astype(bf), 'w3': w3.astype(bf),
        'w4': w4.astype(bf), 'fc5w': fc5w.astype(bf), 'fc6w': fc6w.astype(bf),
        'fc78w': fc78w.astype(bf), 'consts': consts,
    }


_cache = {}


def _get_nc(NB=1024, S=256, debug_taps=False):
    key = (NB, S, debug_taps)
    if key not in _cache:
        _cache[key] = build(NB, S, debug_taps)
    return _cache[key]


def kernel(**inputs):
    from concourse.bass_utils import run_bass_kernel_spmd
    x = np.asarray(inputs['x'], np.float32)
    B = x.shape[0]
    NB = B // NCORES
    xf = x.reshape(B, 128)
    w = prep_weights(inputs)
    nc = _get_nc(NB=NB, S=256)
    in_maps = []
    for c in range(NCORES):
        m = dict(w)
        m['x'] = np.ascontiguousarray(xf[c * NB:(c + 1) * NB])
        in_maps.append(m)
    res = run_bass_kernel_spmd(nc, in_maps, list(range(NCORES)))
    return _assemble(res, B, NB)


def _assemble(res, B, NB):
    out = np.empty((B, NCLS), np.float32)
    for c in range(NCORES):
        yc = np.asarray(res.results[c]['y'], np.float32)   # [8, NB]
        out[c * NB:(c + 1) * NB] = yc.T
    return out


def run_traced(inputs, tmpdir=None):
    """Like kernel() but with NTFF tracing; returns (out, BassKernelResults)."""
    from concourse.bass_utils import run_bass_kernel_spmd
    x = np.asarray(inputs['x'], np.float32)
    B = x.shape[0]
    NB = B // NCORES
    xf = x.reshape(B, 128)
    w = prep_weights(inputs)
    nc = _get_nc(NB=NB, S=256)
    in_maps = []
    for c in range(NCORES):
        m = dict(w)
        m['x'] = np.ascontiguousarray(xf[c * NB:(c + 1) * NB])
        in_maps.append(m)
    res = run_bass_kernel_spmd(nc, in_maps, list(range(NCORES)), trace=True,
                               tmpdir=tmpdir)
    return _assemble(res, B, NB), res
